# revision 6
# baseline (speedup 1.0000x reference)
"""Supervised-contrastive loss (nn_ConLoss) on 8 Trainium2 NeuronCores.

Row-sharded data-parallel: each core computes its [1024, 8192] block of
Z = X X^T with fp8-e4m3 DoubleRow matmuls (2 MACs/cell/cycle), the
vector engine evacuates PSUM with a fused copy+row-max into fp16 SBUF,
then one big-FD exp pass per row block runs on the scalar engine with
sum-accumulate.
Positive-pair sums come algebraically from S = per-class feature sums:
sum_{j:lab_j=lab_i} z_ij = x_i . S_{lab_i} - ||x_i||^2 (tiny matmul +
one-hot dot), so no O(N^2) mask pass exists. Ln and the final per-row
loss math are batched once at the end. Host sums partial losses.
"""
import numpy as np

TEMPERATURE = 0.1
N, D, C = 8192, 512, 8
R = N // C            # 1024 rows per core
NRB = R // 128        # 8 row blocks of 128
CW = 1024             # elementwise tile width (2 psum banks)
NCC = N // CW         # 8 column chunks
NK = D // 128         # 4 k-subtiles
NKP = NK // 2         # 2 DoubleRow k-pairs
NL = 100              # num classes

_NC_CACHE = {}


def _build_nc():
    if "nc" in _NC_CACHE:
        return _NC_CACHE["nc"]
    import concourse.tile as tile
    from concourse import bacc, mybir
    from contextlib import ExitStack

    DT = mybir.dt
    ALU = mybir.AluOpType
    ACTF = mybir.ActivationFunctionType
    DR = mybir.MatmulPerfMode.DoubleRow

    nc = bacc.Bacc("TRN2", target_bir_lowering=False, debug=False)
    xt_d = nc.dram_tensor("xt8", [128, NK, N], DT.float8e4, kind="ExternalInput")
    st_d = nc.dram_tensor("st8", [D, NL], DT.float8e4, kind="ExternalInput")
    lrow_d = nc.dram_tensor("labrow", [128, NRB], DT.float32, kind="ExternalInput")
    negw_d = nc.dram_tensor("negw", [128, NRB], DT.float32, kind="ExternalInput")
    icnt_d = nc.dram_tensor("icnt10", [128, NRB], DT.float32, kind="ExternalInput")
    nrm_d = nc.dram_tensor("nrm", [128, NRB], DT.float32, kind="ExternalInput")
    kidx_d = nc.dram_tensor("kidx", [128, NL], DT.float32, kind="ExternalInput")
    ome_d = nc.dram_tensor("ome", [128, 128], DT.float32, kind="ExternalInput")
    res_d = nc.dram_tensor("res", [128, NRB], DT.float32, kind="ExternalOutput")

    with tile.TileContext(nc) as tc, ExitStack() as ctx:
        xt_pool = ctx.enter_context(tc.tile_pool(name="xt", bufs=1))
        sml_pool = ctx.enter_context(tc.tile_pool(name="sml", bufs=1))
        z_pool = ctx.enter_context(tc.tile_pool(name="z", bufs=2))
        e_pool = ctx.enter_context(tc.tile_pool(name="e", bufs=2))
        ps_pool = ctx.enter_context(tc.tile_pool(name="ps", bufs=3, space="PSUM"))
        g_pool = ctx.enter_context(tc.tile_pool(name="g", bufs=2, space="PSUM"))
        st_pool = ctx.enter_context(tc.tile_pool(name="st", bufs=2))
        acc_pool = ctx.enter_context(tc.tile_pool(name="acc", bufs=1))

        st_sb = []
        for k in range(NK):
            t = sml_pool.tile([128, NL], DT.float8e4, tag=f"st_{k}")
            nc.sync.dma_start(t[:], st_d[k * 128:(k + 1) * 128, :])
            st_sb.append(t)
        lrow_sb = sml_pool.tile([128, NRB], DT.float32)
        nc.sync.dma_start(lrow_sb[:], lrow_d[:])
        negw_sb = sml_pool.tile([128, NRB], DT.float32)
        nc.sync.dma_start(negw_sb[:], negw_d[:])
        icnt_sb = sml_pool.tile([128, NRB], DT.float32)
        nc.sync.dma_start(icnt_sb[:], icnt_d[:])
        nrm_sb = sml_pool.tile([128, NRB], DT.float32)
        nc.sync.dma_start(nrm_sb[:], nrm_d[:])
        kidx_sb = sml_pool.tile([128, NL], DT.float32)
        nc.sync.dma_start(kidx_sb[:], kidx_d[:])
        ome_sb = sml_pool.tile([128, 128], DT.float32)
        nc.sync.dma_start(ome_sb[:], ome_d[:])

        # fp8 xt tiles [128, 2, CW] per (k-pair, chunk); chunk-major order
        xt_sb = {}
        for cc in range(NCC):
            for kp in range(NKP):
                t = xt_pool.tile([128, 2, CW], DT.float8e4, tag=f"xt_{kp}_{cc}")
                nc.sync.dma_start(
                    t[:], xt_d[:, 2 * kp:2 * kp + 2, cc * CW:(cc + 1) * CW])
                xt_sb[kp, cc] = t

        negm_all = acc_pool.tile([128, NRB], DT.float32)
        ssum_all = acc_pool.tile([128, NRB], DT.float32)
        gsel_all = acc_pool.tile([128, NRB], DT.float32)

        z_tiles = []
        for rb in range(NRB):
            rbs = slice(rb * 128, (rb + 1) * 128)

            # G = X_rows . S^T -> [128, 100] psum (normal fp8 matmuls)
            gps = g_pool.tile([128, NL], DT.float32, tag="gps")
            for k in range(NK):
                nc.tensor.matmul(gps[:], xt_sb[k // 2, 0][:, k % 2, rbs],
                                 st_sb[k][:], start=(k == 0), stop=(k == NK - 1))

            rm = st_pool.tile([128, NCC], DT.float32, tag="rm")
            zrb = z_pool.tile([128, N], DT.float16, tag="z")
            z_tiles.append(zrb)

            for cc in range(NCC):
                ps = ps_pool.tile([128, CW], DT.float32, tag="ps")
                for h in range(2):
                    for kp in range(NKP):
                        nc.tensor.matmul(
                            ps[:, h * 512:(h + 1) * 512],
                            xt_sb[kp, 0][:, :, rbs],
                            xt_sb[kp, cc][:, :, h * 512:(h + 1) * 512],
                            start=(kp == 0), stop=(kp == NKP - 1),
                            perf_mode=DR)
                if cc == 0:
                    off = rb * 128
                    nc.vector.scalar_tensor_tensor(
                        out=ps[:, off:off + 128], in0=ps[:, off:off + 128],
                        scalar=0.0, in1=ome_sb[:],
                        op0=ALU.bypass, op1=ALU.mult)
                nc.vector.tensor_scalar(
                    out=zrb[:, cc * CW:(cc + 1) * CW], in0=ps[:],
                    scalar1=0.0, scalar2=-3.0e38,
                    op0=ALU.add, op1=ALU.max, accum_out=rm[:, cc:cc + 1])

            mfin = st_pool.tile([128, 1], DT.float32, tag="mfin")
            nc.vector.tensor_reduce(mfin[:], rm[:], axis=mybir.AxisListType.X,
                                    op=ALU.max)
            nc.vector.tensor_scalar_mul(negm_all[:, rb:rb + 1], mfin[:], -10.0)

            gscr = st_pool.tile([128, NL], DT.float32, tag="gscr")
            nc.vector.scalar_tensor_tensor(
                out=gscr[:], in0=kidx_sb[:], scalar=lrow_sb[:, rb:rb + 1],
                in1=gps[:], op0=ALU.is_equal, op1=ALU.mult,
                accum_out=gsel_all[:, rb:rb + 1])

            escr = e_pool.tile([128, N], DT.float16, tag="escr")
            nc.scalar.activation(
                out=escr[:], in_=zrb[:], func=ACTF.Exp,
                bias=negm_all[:, rb:rb + 1], scale=10.0,
                accum_out=ssum_all[:, rb:rb + 1])

        lns_all = acc_pool.tile([128, NRB], DT.float32)
        nc.scalar.activation(lns_all[:], ssum_all[:], ACTF.Ln)
        t1 = acc_pool.tile([128, NRB], DT.float32)
        nc.vector.tensor_sub(t1[:], gsel_all[:], nrm_sb[:])
        t2 = acc_pool.tile([128, NRB], DT.float32)
        nc.vector.tensor_mul(t2[:], t1[:], icnt_sb[:])
        t3 = acc_pool.tile([128, NRB], DT.float32)
        nc.vector.tensor_sub(t3[:], t2[:], lns_all[:])
        t4 = acc_pool.tile([128, NRB], DT.float32)
        nc.vector.tensor_add(t4[:], t3[:], negm_all[:])
        res_sb = acc_pool.tile([128, NRB], DT.float32)
        nc.vector.tensor_mul(res_sb[:], t4[:], negw_sb[:])
        nc.sync.dma_start(res_d[:], res_sb[:])

    nc.compile()
    _NC_CACHE["nc"] = nc
    return nc


def _reset_device():
    try:
        import ctypes, jax
        jax.devices()
        ctypes.CDLL("/opt/axon/libaxon_pjrt.so").axon_reset()
    except Exception:
        pass


def _make_in_maps(features, labels, weights):
    from concourse import mybir
    f8dt = mybir.dt.np(mybir.dt.float8e4)

    f = np.ascontiguousarray(np.asarray(features, dtype=np.float32))
    lab = np.asarray(labels).astype(np.int32)
    w = np.asarray(weights, dtype=np.float32)

    xt = f.T.astype(np.float32)                          # [D, N]
    hist = np.bincount(lab, minlength=NL).astype(np.float64)
    icnt10_full = (10.0 / (hist[lab] - 1.0)).astype(np.float32)
    nrm_full = (f.astype(np.float64) ** 2).sum(axis=1).astype(np.float32)

    S = np.zeros((NL, D), dtype=np.float64)
    np.add.at(S, lab, f.astype(np.float64))
    st8 = np.ascontiguousarray(S.T.astype(np.float32).astype(f8dt))

    kidx = np.broadcast_to(np.arange(NL, dtype=np.float32)[None, :], (128, NL))
    ome = (1.0 - np.eye(128)).astype(np.float32)

    in_maps = []
    for c in range(C):
        sl = slice(c * R, (c + 1) * R)
        perm = np.concatenate([
            np.arange(c * R, (c + 1) * R),
            np.arange(0, c * R),
            np.arange((c + 1) * R, N),
        ])
        xtp8 = xt[:, perm].astype(f8dt)                  # [D, N] fp8
        # [128, NK, N]: element [p, ks, n] = xtp8[ks*128+p, n]
        xt8 = np.ascontiguousarray(
            xtp8.reshape(NK, 128, N).transpose(1, 0, 2))
        in_maps.append({
            "xt8": xt8,
            "st8": st8,
            "labrow": np.ascontiguousarray(
                lab[sl].astype(np.float32).reshape(NRB, 128).T),
            "negw": np.ascontiguousarray((-w[sl]).reshape(NRB, 128).T),
            "icnt10": np.ascontiguousarray(
                icnt10_full[sl].reshape(NRB, 128).T),
            "nrm": np.ascontiguousarray(nrm_full[sl].reshape(NRB, 128).T),
            "kidx": np.ascontiguousarray(kidx),
            "ome": ome,
        })

    return in_maps


def kernel(features, labels, weights):
    from concourse.bass_utils import run_bass_kernel_spmd

    w = np.asarray(weights, dtype=np.float32)
    nc = _build_nc()
    _reset_device()
    in_maps = _make_in_maps(features, labels, weights)
    out = run_bass_kernel_spmd(nc, in_maps, list(range(C)))
    total = np.float64(0.0)
    for c in range(C):
        total += out.results[c]["res"].astype(np.float64).sum()
    loss = total / np.float64(w.astype(np.float64).sum())
    return np.asarray(loss, dtype=np.float32)


# revision 7
# speedup vs baseline: 1.0040x; 1.0040x over previous
"""Supervised-contrastive loss (nn_ConLoss) on 8 Trainium2 NeuronCores.

Row-sharded data-parallel: each core computes its [1024, 8192] block of
Z = X X^T with fp8-e4m3 DoubleRow matmuls (2 MACs/cell/cycle), the
vector engine evacuates PSUM with a fused copy+row-max into fp16 SBUF,
then one big-FD exp pass per row block runs on the scalar engine with
sum-accumulate.
Positive-pair sums come algebraically from S = per-class feature sums:
sum_{j:lab_j=lab_i} z_ij = x_i . S_{lab_i} - ||x_i||^2 (tiny matmul +
one-hot dot), so no O(N^2) mask pass exists. Ln and the final per-row
loss math are batched once at the end. Host sums partial losses.
"""
import numpy as np

TEMPERATURE = 0.1
N, D, C = 8192, 512, 8
R = N // C            # 1024 rows per core
NRB = R // 128        # 8 row blocks of 128
CW = 1024             # elementwise tile width (2 psum banks)
NCC = N // CW         # 8 column chunks
NK = D // 128         # 4 k-subtiles
NKP = NK // 2         # 2 DoubleRow k-pairs
NL = 100              # num classes

_NC_CACHE = {}


def _build_nc():
    if "nc" in _NC_CACHE:
        return _NC_CACHE["nc"]
    import concourse.tile as tile
    from concourse import bacc, mybir
    from contextlib import ExitStack

    DT = mybir.dt
    ALU = mybir.AluOpType
    ACTF = mybir.ActivationFunctionType
    DR = mybir.MatmulPerfMode.DoubleRow

    nc = bacc.Bacc("TRN2", target_bir_lowering=False, debug=False)
    xt_d = nc.dram_tensor("xt8", [128, NK, N], DT.float8e4, kind="ExternalInput")
    st_d = nc.dram_tensor("st8", [D, NL], DT.float8e4, kind="ExternalInput")
    lrow_d = nc.dram_tensor("labrow", [128, NRB], DT.float32, kind="ExternalInput")
    negw_d = nc.dram_tensor("negw", [128, NRB], DT.float32, kind="ExternalInput")
    icnt_d = nc.dram_tensor("icnt10", [128, NRB], DT.float32, kind="ExternalInput")
    nrm_d = nc.dram_tensor("nrm", [128, NRB], DT.float32, kind="ExternalInput")
    kidx_d = nc.dram_tensor("kidx", [128, NL], DT.float32, kind="ExternalInput")
    ome_d = nc.dram_tensor("ome", [128, 128], DT.float32, kind="ExternalInput")
    res_d = nc.dram_tensor("res", [128, NRB], DT.float32, kind="ExternalOutput")

    with tile.TileContext(nc) as tc, ExitStack() as ctx:
        xt_pool = ctx.enter_context(tc.tile_pool(name="xt", bufs=1))
        sml_pool = ctx.enter_context(tc.tile_pool(name="sml", bufs=1))
        z_pool = ctx.enter_context(tc.tile_pool(name="z", bufs=2))
        e_pool = ctx.enter_context(tc.tile_pool(name="e", bufs=2))
        ps_pool = ctx.enter_context(tc.tile_pool(name="ps", bufs=3, space="PSUM"))
        g_pool = ctx.enter_context(tc.tile_pool(name="g", bufs=2, space="PSUM"))
        st_pool = ctx.enter_context(tc.tile_pool(name="st", bufs=2))
        acc_pool = ctx.enter_context(tc.tile_pool(name="acc", bufs=1))

        st_sb = []
        for k in range(NK):
            t = sml_pool.tile([128, NL], DT.float8e4, tag=f"st_{k}")
            nc.sync.dma_start(t[:], st_d[k * 128:(k + 1) * 128, :])
            st_sb.append(t)
        lrow_sb = sml_pool.tile([128, NRB], DT.float32)
        nc.sync.dma_start(lrow_sb[:], lrow_d[:])
        negw_sb = sml_pool.tile([128, NRB], DT.float32)
        nc.sync.dma_start(negw_sb[:], negw_d[:])
        icnt_sb = sml_pool.tile([128, NRB], DT.float32)
        nc.sync.dma_start(icnt_sb[:], icnt_d[:])
        nrm_sb = sml_pool.tile([128, NRB], DT.float32)
        nc.sync.dma_start(nrm_sb[:], nrm_d[:])
        kidx_sb = sml_pool.tile([128, NL], DT.float32)
        nc.sync.dma_start(kidx_sb[:], kidx_d[:])
        ome_sb = sml_pool.tile([128, 128], DT.float32)
        nc.sync.dma_start(ome_sb[:], ome_d[:])

        # fp8 xt tiles [128, 2, CW] per (k-pair, chunk); chunk-major order
        xt_sb = {}
        for cc in range(NCC):
            for kp in range(NKP):
                t = xt_pool.tile([128, 2, CW], DT.float8e4, tag=f"xt_{kp}_{cc}")
                eng = nc.sync if (cc % 2 == 0) else nc.scalar
                eng.dma_start(
                    t[:], xt_d[:, 2 * kp:2 * kp + 2, cc * CW:(cc + 1) * CW])
                xt_sb[kp, cc] = t

        negm_all = acc_pool.tile([128, NRB], DT.float32)
        ssum_all = acc_pool.tile([128, NRB], DT.float32)
        gsel_all = acc_pool.tile([128, NRB], DT.float32)

        z_tiles = []
        for rb in range(NRB):
            rbs = slice(rb * 128, (rb + 1) * 128)

            # G = X_rows . S^T -> [128, 100] psum (normal fp8 matmuls)
            gps = g_pool.tile([128, NL], DT.float32, tag="gps")
            for k in range(NK):
                nc.tensor.matmul(gps[:], xt_sb[k // 2, 0][:, k % 2, rbs],
                                 st_sb[k][:], start=(k == 0), stop=(k == NK - 1))

            rm = st_pool.tile([128, NCC], DT.float32, tag="rm")
            zrb = z_pool.tile([128, N], DT.float16, tag="z")
            z_tiles.append(zrb)

            for cc in range(NCC):
                ps = ps_pool.tile([128, CW], DT.float32, tag="ps")
                for h in range(2):
                    for kp in range(NKP):
                        nc.tensor.matmul(
                            ps[:, h * 512:(h + 1) * 512],
                            xt_sb[kp, 0][:, :, rbs],
                            xt_sb[kp, cc][:, :, h * 512:(h + 1) * 512],
                            start=(kp == 0), stop=(kp == NKP - 1),
                            perf_mode=DR)
                if cc == 0:
                    off = rb * 128
                    nc.vector.scalar_tensor_tensor(
                        out=ps[:, off:off + 128], in0=ps[:, off:off + 128],
                        scalar=0.0, in1=ome_sb[:],
                        op0=ALU.bypass, op1=ALU.mult)
                nc.vector.tensor_scalar(
                    out=zrb[:, cc * CW:(cc + 1) * CW], in0=ps[:],
                    scalar1=0.0, scalar2=-3.0e38,
                    op0=ALU.add, op1=ALU.max, accum_out=rm[:, cc:cc + 1])
                if rb == NRB - 1 and cc == 3:
                    # first-half exp of the last row block overlaps the
                    # evacuation of its second half (exact: rescaled below)
                    mfa = st_pool.tile([128, 1], DT.float32, tag="mfa")
                    nc.vector.tensor_reduce(mfa[:], rm[:, 0:4],
                                            axis=mybir.AxisListType.X, op=ALU.max)
                    nega = st_pool.tile([128, 1], DT.float32, tag="nega")
                    nc.vector.tensor_scalar_mul(nega[:], mfa[:], -10.0)
                    escr_a = e_pool.tile([128, N], DT.float16, tag="escr")
                    ssa = st_pool.tile([128, 1], DT.float32, tag="ssa")
                    nc.scalar.activation(
                        out=escr_a[:, 0:4 * CW], in_=zrb[:, 0:4 * CW],
                        func=ACTF.Exp, bias=nega[:], scale=10.0,
                        accum_out=ssa[:])

            if rb < NRB - 1:
                mfin = st_pool.tile([128, 1], DT.float32, tag="mfin")
                nc.vector.tensor_reduce(mfin[:], rm[:],
                                        axis=mybir.AxisListType.X, op=ALU.max)
                nc.vector.tensor_scalar_mul(negm_all[:, rb:rb + 1], mfin[:],
                                            -10.0)

            gscr = st_pool.tile([128, NL], DT.float32, tag="gscr")
            nc.vector.scalar_tensor_tensor(
                out=gscr[:], in0=kidx_sb[:], scalar=lrow_sb[:, rb:rb + 1],
                in1=gps[:], op0=ALU.is_equal, op1=ALU.mult,
                accum_out=gsel_all[:, rb:rb + 1])

            if rb < NRB - 1:
                escr = e_pool.tile([128, N], DT.float16, tag="escr")
                nc.scalar.activation(
                    out=escr[:], in_=zrb[:], func=ACTF.Exp,
                    bias=negm_all[:, rb:rb + 1], scale=10.0,
                    accum_out=ssum_all[:, rb:rb + 1])
            else:
                mfb = st_pool.tile([128, 1], DT.float32, tag="mfb")
                nc.vector.tensor_reduce(mfb[:], rm[:, 4:8],
                                        axis=mybir.AxisListType.X, op=ALU.max)
                negb = st_pool.tile([128, 1], DT.float32, tag="negb")
                nc.vector.tensor_scalar_mul(negb[:], mfb[:], -10.0)
                escr_b = e_pool.tile([128, N], DT.float16, tag="escr")
                ssb = st_pool.tile([128, 1], DT.float32, tag="ssb")
                nc.scalar.activation(
                    out=escr_b[:, 4 * CW:8 * CW], in_=zrb[:, 4 * CW:8 * CW],
                    func=ACTF.Exp, bias=negb[:], scale=10.0,
                    accum_out=ssb[:])
                # negm = min(nega, negb) == -10 * rowmax; exact recombine:
                # ssum = ssa*exp(negm-nega) + ssb*exp(negm-negb)
                nc.vector.tensor_tensor(
                    out=negm_all[:, rb:rb + 1], in0=nega[:], in1=negb[:],
                    op=ALU.min)
                da = st_pool.tile([128, 1], DT.float32, tag="da")
                nc.vector.tensor_sub(da[:], negm_all[:, rb:rb + 1], nega[:])
                db = st_pool.tile([128, 1], DT.float32, tag="db")
                nc.vector.tensor_sub(db[:], negm_all[:, rb:rb + 1], negb[:])
                fa = st_pool.tile([128, 1], DT.float32, tag="fa")
                nc.scalar.activation(fa[:], da[:], ACTF.Exp)
                fb = st_pool.tile([128, 1], DT.float32, tag="fb")
                nc.scalar.activation(fb[:], db[:], ACTF.Exp)
                wa = st_pool.tile([128, 1], DT.float32, tag="wa")
                nc.vector.tensor_mul(wa[:], ssa[:], fa[:])
                wb = st_pool.tile([128, 1], DT.float32, tag="wb")
                nc.vector.tensor_mul(wb[:], ssb[:], fb[:])
                nc.vector.tensor_add(ssum_all[:, rb:rb + 1], wa[:], wb[:])

        lns_all = acc_pool.tile([128, NRB], DT.float32)
        nc.scalar.activation(lns_all[:], ssum_all[:], ACTF.Ln)
        t1 = acc_pool.tile([128, NRB], DT.float32)
        nc.vector.tensor_sub(t1[:], gsel_all[:], nrm_sb[:])
        t2 = acc_pool.tile([128, NRB], DT.float32)
        nc.vector.tensor_mul(t2[:], t1[:], icnt_sb[:])
        t3 = acc_pool.tile([128, NRB], DT.float32)
        nc.vector.tensor_sub(t3[:], t2[:], lns_all[:])
        t4 = acc_pool.tile([128, NRB], DT.float32)
        nc.vector.tensor_add(t4[:], t3[:], negm_all[:])
        res_sb = acc_pool.tile([128, NRB], DT.float32)
        nc.vector.tensor_mul(res_sb[:], t4[:], negw_sb[:])
        nc.sync.dma_start(res_d[:], res_sb[:])

    nc.compile()
    _NC_CACHE["nc"] = nc
    return nc


def _reset_device():
    try:
        import ctypes, jax
        jax.devices()
        ctypes.CDLL("/opt/axon/libaxon_pjrt.so").axon_reset()
    except Exception:
        pass


def _make_in_maps(features, labels, weights):
    from concourse import mybir
    f8dt = mybir.dt.np(mybir.dt.float8e4)

    f = np.ascontiguousarray(np.asarray(features, dtype=np.float32))
    lab = np.asarray(labels).astype(np.int32)
    w = np.asarray(weights, dtype=np.float32)

    xt = f.T.astype(np.float32)                          # [D, N]
    hist = np.bincount(lab, minlength=NL).astype(np.float64)
    icnt10_full = (10.0 / (hist[lab] - 1.0)).astype(np.float32)
    nrm_full = (f.astype(np.float64) ** 2).sum(axis=1).astype(np.float32)

    S = np.zeros((NL, D), dtype=np.float64)
    np.add.at(S, lab, f.astype(np.float64))
    st8 = np.ascontiguousarray(S.T.astype(np.float32).astype(f8dt))

    kidx = np.broadcast_to(np.arange(NL, dtype=np.float32)[None, :], (128, NL))
    ome = (1.0 - np.eye(128)).astype(np.float32)

    in_maps = []
    for c in range(C):
        sl = slice(c * R, (c + 1) * R)
        perm = np.concatenate([
            np.arange(c * R, (c + 1) * R),
            np.arange(0, c * R),
            np.arange((c + 1) * R, N),
        ])
        xtp8 = xt[:, perm].astype(f8dt)                  # [D, N] fp8
        # [128, NK, N]: element [p, ks, n] = xtp8[ks*128+p, n]
        xt8 = np.ascontiguousarray(
            xtp8.reshape(NK, 128, N).transpose(1, 0, 2))
        in_maps.append({
            "xt8": xt8,
            "st8": st8,
            "labrow": np.ascontiguousarray(
                lab[sl].astype(np.float32).reshape(NRB, 128).T),
            "negw": np.ascontiguousarray((-w[sl]).reshape(NRB, 128).T),
            "icnt10": np.ascontiguousarray(
                icnt10_full[sl].reshape(NRB, 128).T),
            "nrm": np.ascontiguousarray(nrm_full[sl].reshape(NRB, 128).T),
            "kidx": np.ascontiguousarray(kidx),
            "ome": ome,
        })

    return in_maps


def kernel(features, labels, weights):
    from concourse.bass_utils import run_bass_kernel_spmd

    w = np.asarray(weights, dtype=np.float32)
    nc = _build_nc()
    _reset_device()
    in_maps = _make_in_maps(features, labels, weights)
    out = run_bass_kernel_spmd(nc, in_maps, list(range(C)))
    total = np.float64(0.0)
    for c in range(C):
        total += out.results[c]["res"].astype(np.float64).sum()
    loss = total / np.float64(w.astype(np.float64).sum())
    return np.asarray(loss, dtype=np.float32)
